# revision 3
# baseline (speedup 1.0000x reference)
"""Distributed causal multi-head attention + output projection for TRN2 (8 NeuronCores).

Problem: q,k,v [4, 2048, 1024] f32, W [1024, 1024], b zeros, mask zeros (no padding).
  out = proj(softmax(causal(q@k.T/8)) @ v) @ W.T + b

Sharding: head-parallel attention + token-parallel projection, glued by an
8-way AllToAll of the attention outputs (bf16).
  - Core c computes attention for heads {2c, 2c+1} over all 4 batches
    (8 (batch, head) units/core, identical causal structure on every core -> SPMD-uniform).
  - Attention outputs (normalized, bf16) land in AllToAll input bounces laid
    out as [8 token-slices, rows, 128 head-dims].
  - AllToAll gives each core all 1024 feature dims for its 1024-token slice.
  - Each core projects its tokens with the (replicated) W and writes
    out[1024, 1024] f32; the host concatenates the 8 slices.

Pipelining: attention runs in two phases — phase 0 produces rows 0:512 of
every token slice (q-blocks 0 and 2 of each unit), phase 1 rows 512:1024
(q-blocks 1 and 3). Each phase feeds its own AllToAll + projection chunk, so
the first exchange and half the projection overlap phase-1 attention.

Compute: QK/AV/projection on TensorE in bf16 (f32 PSUM accumulation), exp on
ScalarE (softmax without max-subtraction: scores ~ N(0,1), exp is safe in
f32), causal handled at tile granularity (strictly-above-diagonal tiles never
computed; diagonal 128x128 tiles masked multiplicatively after exp). Softmax
denominator comes free from a ones-column baked into the v shard layout.
"""

import sys

sys.path.insert(0, "/opt/trn_rl_repo")

import numpy as np
import ml_dtypes

import concourse.bass as bass  # noqa: F401
import concourse.mybir as mybir
from concourse import bacc
from concourse.bass_utils import run_bass_kernel_spmd
from concourse.tile import TileContext
from concourse.masks import make_upper_triangular

B, S, D, H, DH = 4, 2048, 1024, 16, 64
P = 128
NCORES = 8
UNITS = 8          # (batch, local head) pairs per core
QBLK = 512         # q columns per score block
NQB = S // QBLK    # 4
NKC = S // P       # 16 key chunks
TOK = (B * S) // NCORES  # 1024 tokens projected per core
HTOK = TOK // 2    # 512 token rows per exchange chunk

BF16 = ml_dtypes.bfloat16

_CACHE = {}


def _build():
    bf = mybir.dt.bfloat16
    f32 = mybir.dt.float32
    Exp = mybir.ActivationFunctionType.Exp

    nc = bacc.Bacc("TRN2", target_bir_lowering=False, debug=False, num_devices=NCORES)

    # kTz: [unit, 128, S]; each unit's k^T occupies the same 64-partition range
    # as its q in the pair-packed q tile (zeros elsewhere), so a K=128
    # contraction selects exactly that head.
    kT_ext = nc.declare_dram_parameter("kTz", [UNITS, P, S], bf, isOutput=False)
    # qT: [pair(=batch), 128, S]; partitions 0:64 = head 2c, 64:128 = head 2c+1.
    qT_ext = nc.declare_dram_parameter("qT", [UNITS // 2, P, S], bf, isOutput=False)
    # v: [unit, 128, 16*65]; chunk kc holds [v_head[kc*128+p, 0:64], 1.0] —
    # the ones column makes AV emit the softmax denominator for free.
    v_ext = nc.declare_dram_parameter("v", [UNITS, P, NKC * (DH + 1)], bf, isOutput=False)
    # wT = W.T (contraction dim major): [1024 d, 1024 o].
    wT_ext = nc.declare_dram_parameter("wT", [D, D], bf, isOutput=False)
    out_ext = nc.declare_dram_parameter("out", [TOK, D], f32, isOutput=True)

    with TileContext(nc) as tc:
        with (
            tc.tile_pool(name="const", bufs=1) as constp,
            tc.tile_pool(name="q", bufs=4) as qp,
            tc.tile_pool(name="k", bufs=8) as kp,
            tc.tile_pool(name="v", bufs=8) as vp,
            tc.tile_pool(name="attn", bufs=10) as attnp,
            tc.tile_pool(name="anorm", bufs=6) as anp,
            tc.tile_pool(name="astage", bufs=4) as astp,
            tc.tile_pool(name="at", bufs=2) as atp,
            tc.tile_pool(name="w", bufs=1) as wp,
            tc.tile_pool(name="osb", bufs=2) as osb,
            tc.tile_pool(name="dram", bufs=1, space="DRAM") as dramp,
            tc.tile_pool(name="pscore", bufs=2, space="PSUM") as pscore,
            tc.tile_pool(name="pav", bufs=2, space="PSUM") as pav,
            tc.tile_pool(name="pproj", bufs=2, space="PSUM") as pproj,
        ):
            # Multiplicative causal mask for diagonal tiles, [k, q] layout:
            # m01[kk, qq] = 1.0 iff qq >= kk.
            m01 = constp.tile([P, P], bf)
            make_upper_triangular(nc, m01[:], val=1.0, diag=True)

            w_sb = wp.tile([P, D // P, D], bf)
            nc.sync.dma_start(w_sb[:], wT_ext.ap().rearrange("(dc p) o -> p dc o", p=P))

            # Resident q/k/v for all units (loaded once, used by both phases).
            qts, kts, vts = [], [], []
            for b_ in range(B):
                qt2 = qp.tile([P, S], bf, tag="q", name=f"qt{b_}")
                nc.sync.dma_start(qt2[:], qT_ext.ap()[b_])
                qts.append(qt2)
            for u in range(UNITS):
                kt = kp.tile([P, S], bf, tag="k", name=f"kt{u}")
                nc.sync.dma_start(kt[:], kT_ext.ap()[u])
                kts.append(kt)
                vt = vp.tile([P, NKC, DH + 1], bf, tag="v", name=f"vt{u}")
                nc.sync.dma_start(
                    vt[:], v_ext.ap()[u].rearrange("p (c d) -> p c d", d=DH + 1)
                )
                vts.append(vt)

            # Exchange bounces, one pair per token-half chunk.
            a2a_in = [dramp.tile([NCORES, HTOK, P], bf, name=f"a2a_in{i}") for i in range(2)]
            a2a_out = [dramp.tile([NCORES, HTOK, P], bf, name=f"a2a_out{i}") for i in range(2)]

            def attention_block(u, qb):
                """Scores+softmax+AV for unit u, q-block qb; stage A rows to bounce."""
                b_, hi = u // 2, u % 2
                qt2, kt, vt = qts[b_], kts[u], vts[u]
                npairs = 2 * qb + 2  # key-chunk pairs covering kc 0 .. 4qb+3
                attn_tiles = []
                for g in range(npairs):
                    ps = pscore.tile([P, 2, QBLK], f32, tag="ps")
                    at = attnp.tile([P, 2, QBLK], bf, tag="attn")
                    for r in range(2):
                        kc = 2 * g + r
                        i = kc - 4 * qb  # >= 0 only inside the diagonal block
                        off = i * P if i > 0 else 0
                        nc.tensor.matmul(
                            ps[:, r, off:QBLK],
                            lhsT=kt[:, kc * P : (kc + 1) * P],
                            rhs=qt2[:, qb * QBLK + off : (qb + 1) * QBLK],
                            start=True,
                            stop=True,
                        )
                    nc.scalar.activation(at[:], ps[:], Exp, scale=0.125)
                    for r in range(2):
                        kc = 2 * g + r
                        i = kc - 4 * qb
                        if i >= 0:
                            sl = at[:, r, i * P : (i + 1) * P]
                            nc.vector.tensor_mul(sl, sl, m01[:])
                    attn_tiles.append(at)

                stage = astp.tile([P, 4, DH], bf, tag="astage")
                for j in range(4):
                    qt_g = 4 * qb + j
                    nkc = qt_g + 1
                    po = pav.tile([P, DH + 1], f32, tag="pav")
                    for kc in range(nkc):
                        g, r = kc // 2, kc % 2
                        nc.tensor.matmul(
                            po[:],
                            lhsT=attn_tiles[g][:, r, j * P : (j + 1) * P],
                            rhs=vt[:, kc, :],
                            start=(kc == 0),
                            stop=(kc == nkc - 1),
                        )
                    rec = anp.tile([P, 1], f32, tag="rec")
                    nc.vector.reciprocal(rec[:], po[:, DH : DH + 1])
                    nc.vector.tensor_scalar_mul(stage[:, j, :], po[:, 0:DH], rec[:])
                # qb0 -> chunk0 even slice; qb1 -> chunk1 even; qb2 -> chunk0 odd;
                # qb3 -> chunk1 odd. One DMA per (unit, q-block).
                chunk = qb % 2
                sl = b_ * 2 + (qb // 2)
                dest = a2a_in[chunk][sl, :, hi * DH : (hi + 1) * DH]
                nc.sync.dma_start(dest.rearrange("(c p) d -> p c d", p=P), stage[:])

            def project_chunk(chunk):
                """Project token rows [chunk*512, chunk*512+512) from a2a_out[chunk]."""
                at_c = atp.tile([P, D // P, HTOK], bf, tag="at")
                for dc in range(D // P):
                    nc.sync.dma_start_transpose(at_c[:, dc, :], a2a_out[chunk][dc])
                for tl in range(HTOK // P):
                    tt = chunk * (HTOK // P) + tl
                    ot = osb.tile([P, D], f32, tag="osb")
                    for oc in range(2):
                        pp = pproj.tile([P, 512], f32, tag="pp")
                        for dc in range(D // P):
                            nc.tensor.matmul(
                                pp[:],
                                lhsT=at_c[:, dc, tl * P : (tl + 1) * P],
                                rhs=w_sb[:, dc, oc * 512 : (oc + 1) * 512],
                                start=(dc == 0),
                                stop=(dc == D // P - 1),
                            )
                        nc.vector.tensor_copy(ot[:, oc * 512 : (oc + 1) * 512], pp[:])
                    nc.sync.dma_start(out_ext.ap()[tt * P : (tt + 1) * P, :], ot[:])

            def exchange(chunk):
                nc.gpsimd.collective_compute(
                    "AllToAll",
                    mybir.AluOpType.bypass,
                    replica_groups=[list(range(NCORES))],
                    ins=[a2a_in[chunk].opt()],
                    outs=[a2a_out[chunk].opt()],
                )

            # Phase 0: rows 0:512 of every slice -> exchange -> (overlapped) proj.
            for u in range(UNITS):
                attention_block(u, 0)
                attention_block(u, 2)
            exchange(0)
            # Phase 1 attention overlaps exchange(0) + project_chunk(0).
            project_chunk(0)
            for u in range(UNITS):
                attention_block(u, 1)
                attention_block(u, 3)
            exchange(1)
            project_chunk(1)

    nc.compile()
    return nc


def _shard_inputs(q, k, v):
    """Build the 8 per-core input maps (bf16, attention-friendly layouts)."""
    qh = np.ascontiguousarray(q.reshape(B, S, H, DH))
    kh = np.ascontiguousarray(k.reshape(B, S, H, DH))
    vh = np.ascontiguousarray(v.reshape(B, S, H, DH))
    in_maps = []
    for c in range(NCORES):
        qT = np.zeros((UNITS // 2, P, S), dtype=BF16)
        kTz = np.zeros((UNITS, P, S), dtype=BF16)
        vv = np.empty((UNITS, P, NKC, DH + 1), dtype=BF16)
        vv[:, :, :, DH] = 1.0
        for b_ in range(B):
            for hi in range(2):
                h = 2 * c + hi
                u = b_ * 2 + hi
                qT[b_, hi * DH : (hi + 1) * DH, :] = qh[b_, :, h, :].T.astype(BF16)
                kTz[u, hi * DH : (hi + 1) * DH, :] = kh[b_, :, h, :].T.astype(BF16)
                vv[u, :, :, 0:DH] = (
                    vh[b_, :, h, :].reshape(NKC, P, DH).transpose(1, 0, 2).astype(BF16)
                )
        in_maps.append(
            {"qT": qT, "kTz": kTz, "v": vv.reshape(UNITS, P, NKC * (DH + 1))}
        )
    return in_maps


def _run(q, k, v, W, trace=False):
    if "nc" not in _CACHE:
        _CACHE["nc"] = _build()
    nc = _CACHE["nc"]
    in_maps = _shard_inputs(q, k, v)
    wT = np.ascontiguousarray(W.T).astype(BF16)
    for m in in_maps:
        m["wT"] = wT
    res = run_bass_kernel_spmd(nc, in_maps, core_ids=list(range(NCORES)), trace=trace)
    out = np.empty((B, S, D), dtype=np.float32)
    for c in range(NCORES):
        b_, half = c // 2, c % 2
        out[b_, half * TOK : (half + 1) * TOK, :] = res.results[c]["out"]
    return out, res


def kernel(q, k, v, W, b, mask):
    q = np.asarray(q, dtype=np.float32)
    k = np.asarray(k, dtype=np.float32)
    v = np.asarray(v, dtype=np.float32)
    W = np.asarray(W, dtype=np.float32)
    # b is spec'd all-zero and mask all-zero (no padded keys); the causal mask
    # is applied on-device.
    out, _ = _run(q, k, v, W, trace=False)
    return out


def kernel_profiled(q, k, v, W, b, mask):
    out, res = _run(
        np.asarray(q, np.float32),
        np.asarray(k, np.float32),
        np.asarray(v, np.float32),
        np.asarray(W, np.float32),
        trace=True,
    )
    return out, res


# revision 4
# speedup vs baseline: 1.0687x; 1.0687x over previous
"""Distributed causal multi-head attention + output projection for TRN2 (8 NeuronCores).

Problem: q,k,v [4, 2048, 1024] f32, W [1024, 1024], b zeros, mask zeros (no padding).
  out = proj(softmax(causal(q@k.T/8)) @ v) @ W.T + b

Sharding: head-parallel attention + token-parallel projection, glued by an
8-way AllToAll of the attention outputs (bf16).
  - Core c computes attention for heads {2c, 2c+1} over all 4 batches
    (8 (batch, head) units/core, identical causal structure on every core -> SPMD-uniform).
  - Attention outputs (normalized, bf16) land in AllToAll input bounces laid
    out as [8 token-slices, rows, 128 head-dims].
  - AllToAll gives each core all 1024 feature dims for its 1024-token slice.
  - Each core projects its tokens with the (replicated) W and writes
    out[1024, 1024] f32; the host concatenates the 8 slices.

Pipelining: attention runs in two phases — phase 0 produces rows 0:512 of
every token slice (q-blocks 0 and 2 of each unit), phase 1 rows 512:1024
(q-blocks 1 and 3). Each phase feeds its own AllToAll + projection chunk, so
the first exchange and half the projection overlap phase-1 attention.

Compute: QK/AV/projection on TensorE in bf16 (f32 PSUM accumulation), exp on
ScalarE (softmax without max-subtraction: scores ~ N(0,1), exp is safe in
f32), causal handled at tile granularity (strictly-above-diagonal tiles never
computed; diagonal 128x128 tiles masked multiplicatively after exp). Softmax
denominator comes free from a ones-column baked into the v shard layout.
"""

import sys

sys.path.insert(0, "/opt/trn_rl_repo")

import numpy as np
import ml_dtypes

import concourse.bass as bass  # noqa: F401
import concourse.mybir as mybir
from concourse import bacc
from concourse.bass_utils import run_bass_kernel_spmd
from concourse.tile import TileContext
from concourse.masks import make_upper_triangular

B, S, D, H, DH = 4, 2048, 1024, 16, 64
P = 128
NCORES = 8
UNITS = 8          # (batch, local head) pairs per core
QBLK = 512         # q columns per score block
NQB = S // QBLK    # 4
NKC = S // P       # 16 key chunks
TOK = (B * S) // NCORES  # 1024 tokens projected per core
HTOK = TOK // 2    # 512 token rows per exchange chunk

BF16 = ml_dtypes.bfloat16

_CACHE = {}


def _build():
    bf = mybir.dt.bfloat16
    f32 = mybir.dt.float32
    Exp = mybir.ActivationFunctionType.Exp

    nc = bacc.Bacc("TRN2", target_bir_lowering=False, debug=False, num_devices=NCORES)

    # kTz: [unit, 128, S]; each unit's k^T occupies the same 64-partition range
    # as its q in the pair-packed q tile (zeros elsewhere), so a K=128
    # contraction selects exactly that head.
    kT_ext = nc.declare_dram_parameter("kTz", [UNITS, P, S], bf, isOutput=False)
    # qT: [pair(=batch), 128, S]; partitions 0:64 = head 2c, 64:128 = head 2c+1.
    qT_ext = nc.declare_dram_parameter("qT", [UNITS // 2, P, S], bf, isOutput=False)
    # v: [unit, 128, 16*65]; chunk kc holds [v_head[kc*128+p, 0:64], 1.0] —
    # the ones column makes AV emit the softmax denominator for free.
    v_ext = nc.declare_dram_parameter("v", [UNITS, P, NKC * (DH + 1)], bf, isOutput=False)
    # wT = W.T (contraction dim major): [1024 d, 1024 o].
    wT_ext = nc.declare_dram_parameter("wT", [D, D], bf, isOutput=False)
    out_ext = nc.declare_dram_parameter("out", [TOK, D], f32, isOutput=True)

    with TileContext(nc) as tc:
        with (
            tc.tile_pool(name="const", bufs=1) as constp,
            tc.tile_pool(name="q", bufs=4) as qp,
            tc.tile_pool(name="k", bufs=8) as kp,
            tc.tile_pool(name="v", bufs=8) as vp,
            tc.tile_pool(name="attn", bufs=10) as attnp,
            tc.tile_pool(name="anorm", bufs=6) as anp,
            tc.tile_pool(name="astage", bufs=4) as astp,
            tc.tile_pool(name="at", bufs=2) as atp,
            tc.tile_pool(name="w", bufs=1) as wp,
            tc.tile_pool(name="osb", bufs=2) as osb,
            tc.tile_pool(name="dram", bufs=1, space="DRAM") as dramp,
            tc.tile_pool(name="pscore", bufs=2, space="PSUM") as pscore,
            tc.tile_pool(name="pav", bufs=2, space="PSUM") as pav,
            tc.tile_pool(name="pproj", bufs=2, space="PSUM") as pproj,
        ):
            # Multiplicative causal mask for diagonal tiles, [k, q] layout:
            # m01[kk, qq] = 1.0 iff qq >= kk.
            m01 = constp.tile([P, P], bf)
            make_upper_triangular(nc, m01[:], val=1.0, diag=True)

            w_sb = wp.tile([P, D // P, D], bf)
            nc.sync.dma_start(w_sb[:], wT_ext.ap().rearrange("(dc p) o -> p dc o", p=P))

            # Resident q/k/v for all units (loaded once, used by both phases).
            qts, kts, vts = [], [], []
            for b_ in range(B):
                qt2 = qp.tile([P, S], bf, tag="q", name=f"qt{b_}")
                nc.sync.dma_start(qt2[:], qT_ext.ap()[b_])
                qts.append(qt2)
            for u in range(UNITS):
                kt = kp.tile([P, S], bf, tag="k", name=f"kt{u}")
                nc.sync.dma_start(kt[:], kT_ext.ap()[u])
                kts.append(kt)
                vt = vp.tile([P, NKC, DH + 1], bf, tag="v", name=f"vt{u}")
                nc.sync.dma_start(
                    vt[:], v_ext.ap()[u].rearrange("p (c d) -> p c d", d=DH + 1)
                )
                vts.append(vt)

            # Exchange bounces, one pair per token-half chunk.
            a2a_in = [dramp.tile([NCORES, HTOK, P], bf, name=f"a2a_in{i}") for i in range(2)]
            a2a_out = [dramp.tile([NCORES, HTOK, P], bf, name=f"a2a_out{i}") for i in range(2)]

            def attention_block(u, qb):
                """Scores+softmax+AV for unit u, q-block qb; stage A rows to bounce."""
                b_, hi = u // 2, u % 2
                qt2, kt, vt = qts[b_], kts[u], vts[u]
                npairs = 2 * qb + 2  # key-chunk pairs covering kc 0 .. 4qb+3
                attn_tiles = []
                for g in range(npairs):
                    ps = pscore.tile([P, 2, QBLK], f32, tag="ps")
                    at = attnp.tile([P, 2, QBLK], bf, tag="attn")
                    for r in range(2):
                        kc = 2 * g + r
                        i = kc - 4 * qb  # >= 0 only inside the diagonal block
                        off = i * P if i > 0 else 0
                        nc.tensor.matmul(
                            ps[:, r, off:QBLK],
                            lhsT=kt[:, kc * P : (kc + 1) * P],
                            rhs=qt2[:, qb * QBLK + off : (qb + 1) * QBLK],
                            start=True,
                            stop=True,
                        )
                    nc.scalar.activation(at[:], ps[:], Exp, scale=0.125)
                    for r in range(2):
                        kc = 2 * g + r
                        i = kc - 4 * qb
                        if i >= 0:
                            sl = at[:, r, i * P : (i + 1) * P]
                            nc.vector.tensor_mul(sl, sl, m01[:])
                    attn_tiles.append(at)

                stage = astp.tile([P, 4, DH], bf, tag="astage")
                for j in range(4):
                    qt_g = 4 * qb + j
                    nkc = qt_g + 1
                    po = pav.tile([P, DH + 1], f32, tag="pav")
                    for kc in range(nkc):
                        g, r = kc // 2, kc % 2
                        nc.tensor.matmul(
                            po[:],
                            lhsT=attn_tiles[g][:, r, j * P : (j + 1) * P],
                            rhs=vt[:, kc, :],
                            start=(kc == 0),
                            stop=(kc == nkc - 1),
                        )
                    rec = anp.tile([P, 1], f32, tag="rec")
                    nc.vector.reciprocal(rec[:], po[:, DH : DH + 1])
                    nc.vector.tensor_scalar_mul(stage[:, j, :], po[:, 0:DH], rec[:])
                # qb0 -> chunk0 even slice; qb1 -> chunk1 even; qb2 -> chunk0 odd;
                # qb3 -> chunk1 odd. One DMA per (unit, q-block).
                chunk = qb % 2
                sl = b_ * 2 + (qb // 2)
                dest = a2a_in[chunk][sl, :, hi * DH : (hi + 1) * DH]
                nc.sync.dma_start(dest.rearrange("(c p) d -> p c d", p=P), stage[:])

            def project_chunk(chunk):
                """Project token rows [chunk*512, chunk*512+512) from a2a_out[chunk]."""
                at_c = atp.tile([P, D // P, HTOK], bf, tag="at")
                for dc in range(D // P):
                    nc.sync.dma_start_transpose(at_c[:, dc, :], a2a_out[chunk][dc])
                for tl in range(HTOK // P):
                    tt = chunk * (HTOK // P) + tl
                    ot = osb.tile([P, D], f32, tag="osb")
                    for oc in range(2):
                        pp = pproj.tile([P, 512], f32, tag="pp")
                        for dc in range(D // P):
                            nc.tensor.matmul(
                                pp[:],
                                lhsT=at_c[:, dc, tl * P : (tl + 1) * P],
                                rhs=w_sb[:, dc, oc * 512 : (oc + 1) * 512],
                                start=(dc == 0),
                                stop=(dc == D // P - 1),
                            )
                        nc.vector.tensor_copy(ot[:, oc * 512 : (oc + 1) * 512], pp[:])
                    nc.sync.dma_start(out_ext.ap()[tt * P : (tt + 1) * P, :], ot[:])

            def exchange(chunk):
                nc.gpsimd.collective_compute(
                    "AllToAll",
                    mybir.AluOpType.bypass,
                    replica_groups=[list(range(NCORES))],
                    ins=[a2a_in[chunk].opt()],
                    outs=[a2a_out[chunk].opt()],
                )

            # Phase 0: rows 0:512 of every slice -> exchange -> (overlapped) proj.
            for u in range(UNITS):
                attention_block(u, 0)
                attention_block(u, 2)
            exchange(0)
            # Phase 1 attention overlaps exchange(0); proj chunk 0 is emitted
            # mid-phase so the in-order TensorE stream reaches it only after
            # the exchange has completed (emitting it earlier stalls PE).
            for u in range(UNITS):
                attention_block(u, 1)
                attention_block(u, 3)
                if u == 5:
                    project_chunk(0)
            exchange(1)
            project_chunk(1)

    nc.compile()
    return nc


def _shard_inputs(q, k, v):
    """Build the 8 per-core input maps (bf16, attention-friendly layouts)."""
    qh = np.ascontiguousarray(q.reshape(B, S, H, DH))
    kh = np.ascontiguousarray(k.reshape(B, S, H, DH))
    vh = np.ascontiguousarray(v.reshape(B, S, H, DH))
    in_maps = []
    for c in range(NCORES):
        qT = np.zeros((UNITS // 2, P, S), dtype=BF16)
        kTz = np.zeros((UNITS, P, S), dtype=BF16)
        vv = np.empty((UNITS, P, NKC, DH + 1), dtype=BF16)
        vv[:, :, :, DH] = 1.0
        for b_ in range(B):
            for hi in range(2):
                h = 2 * c + hi
                u = b_ * 2 + hi
                qT[b_, hi * DH : (hi + 1) * DH, :] = qh[b_, :, h, :].T.astype(BF16)
                kTz[u, hi * DH : (hi + 1) * DH, :] = kh[b_, :, h, :].T.astype(BF16)
                vv[u, :, :, 0:DH] = (
                    vh[b_, :, h, :].reshape(NKC, P, DH).transpose(1, 0, 2).astype(BF16)
                )
        in_maps.append(
            {"qT": qT, "kTz": kTz, "v": vv.reshape(UNITS, P, NKC * (DH + 1))}
        )
    return in_maps


def _run(q, k, v, W, trace=False):
    if "nc" not in _CACHE:
        _CACHE["nc"] = _build()
    nc = _CACHE["nc"]
    in_maps = _shard_inputs(q, k, v)
    wT = np.ascontiguousarray(W.T).astype(BF16)
    for m in in_maps:
        m["wT"] = wT
    res = run_bass_kernel_spmd(nc, in_maps, core_ids=list(range(NCORES)), trace=trace)
    out = np.empty((B, S, D), dtype=np.float32)
    for c in range(NCORES):
        b_, half = c // 2, c % 2
        out[b_, half * TOK : (half + 1) * TOK, :] = res.results[c]["out"]
    return out, res


def kernel(q, k, v, W, b, mask):
    q = np.asarray(q, dtype=np.float32)
    k = np.asarray(k, dtype=np.float32)
    v = np.asarray(v, dtype=np.float32)
    W = np.asarray(W, dtype=np.float32)
    # b is spec'd all-zero and mask all-zero (no padded keys); the causal mask
    # is applied on-device.
    out, _ = _run(q, k, v, W, trace=False)
    return out


def kernel_profiled(q, k, v, W, b, mask):
    out, res = _run(
        np.asarray(q, np.float32),
        np.asarray(k, np.float32),
        np.asarray(v, np.float32),
        np.asarray(W, np.float32),
        trace=True,
    )
    return out, res
